# revision 20
# baseline (speedup 1.0000x reference)
"""Trainium2 Bass kernel for nn_Attention (dense transformer block:
qkv proj + RoPE + causal attention + out proj), tensor-parallel over
8 NeuronCores: core c handles batch b=c//2, head-group g=c%2 (8 heads).

Self-contained: hardcodes all shapes; host preps transposed/permuted
shards, device computes partial y per core, host sums head-group pairs
and adds the output bias.

All matmul operands are bf16 (1 cyc/row at any width, half the HBM
traffic; accumulation stays fp32 in PSUM, softmax denominators and the
final output stay fp32).  The RoPE partition swap is a permutation
matmul on the tensor engine.  Weights are pre-arranged on the host so
every weight load is one large contiguous DMA.

This version overlaps the three phases: all SBUF tiles are split by
sequence half (qk0/qk1, vsb0/vsb1) and by q-half (ot0/ot1) so the Tile
scheduler's dependency tracking lets attention on the first 1024
q-columns run concurrently with the second half's projections, and the
output projection of the first q-half runs inside the (ACT-bound)
second-half attention.  The scalar engine does exp only; all PSUM
evictions run on the vector engine.  PSUM is replanned so each set of
concurrently-live pools fits in 8 banks.
"""

from contextlib import ExitStack

import numpy as np

import concourse.bass as bass
import concourse.tile as tile
from concourse import bacc, mybir
from concourse.bass import ds, ts
from concourse.bass_utils import run_bass_kernel_spmd

B, S, D, H, DH = 4, 2048, 1024, 16, 64
HL = 8          # heads per core
INNER = H * DH  # 1024
KC = D // 128   # 8 contraction chunks
NT = S // 128   # 16 token tiles
F32 = mybir.dt.float32
BF16 = mybir.dt.bfloat16


def _pieces(cw):
    """split a psum-tile column span into single-bank matmul pieces"""
    out = [(i * 512, 512) for i in range(cw // 512)]
    if cw % 512:
        out.append((cw - cw % 512, cw % 512))
    return out


def build_kernel(nc, phases=3, loop_n=0):
    xT = nc.dram_tensor("xT", [D, S], BF16, kind="ExternalInput").ap()
    # [t, p, k, n]: per q/k col-tile t, contraction chunk k pre-split
    wqk = nc.dram_tensor("wqk", [8, 128, KC, 128], BF16,
                         kind="ExternalInput").ap()
    wv = nc.dram_tensor("wv", [128, KC, 512], BF16, kind="ExternalInput").ap()
    wo = nc.dram_tensor("wo", [128, 4, D], BF16, kind="ExternalInput").ap()
    cc = nc.dram_tensor("cc", [128, S], BF16, kind="ExternalInput").ap()
    ssw = nc.dram_tensor("ssw", [128, S], BF16, kind="ExternalInput").ap()
    perm = nc.dram_tensor("perm", [128, 128], BF16, kind="ExternalInput").ap()
    y = nc.dram_tensor("y", [S, D], F32, kind="ExternalOutput").ap()

    EXP = mybir.ActivationFunctionType.Exp
    SCALE = 1.0 / np.sqrt(DH)

    with tile.TileContext(nc) as tc, ExitStack() as top:
        if loop_n:
            top.enter_context(tc.For_i(0, loop_n, 1,
                                       hint_engines=(mybir.EngineType.PE,)))
        # ---- long-lived SBUF pools -----------------------------------
        consts = top.enter_context(tc.tile_pool(name="consts", bufs=1))
        wop = top.enter_context(tc.tile_pool(name="wop", bufs=1))
        xtp = top.enter_context(tc.tile_pool(name="xtp", bufs=2))
        wsl = top.enter_context(tc.tile_pool(name="wsl", bufs=2))
        rtmp = top.enter_context(tc.tile_pool(name="rtmp", bufs=3))
        qkp = top.enter_context(tc.tile_pool(name="qkt", bufs=1))
        vpool = top.enter_context(tc.tile_pool(name="vpool", bufs=1))
        opool = top.enter_context(tc.tile_pool(name="opool", bufs=1))
        ppool = top.enter_context(tc.tile_pool(name="ppool", bufs=10))
        lpool = top.enter_context(tc.tile_pool(name="lpool", bufs=2))
        ypool = top.enter_context(tc.tile_pool(name="ypool", bufs=4))

        cc_sb = consts.tile([128, S], BF16, tag="cc", name="cc")
        ssw_sb = consts.tile([128, S], BF16, tag="ssw", name="ssw")
        perm_sb = consts.tile([128, 128], BF16, tag="perm", name="perm")
        wv_sb = consts.tile([128, KC, 512], BF16, tag="wv", name="wv")
        wo_sb = wop.tile([128, 4, D], BF16, tag="wo", name="wo")

        # per-half q/k tiles: [128 (2 heads x 64 rope dims), 1024 tokens]
        qk = [[qkp.tile([128, 1024], BF16, tag=f"qk{hf}t{t}",
                        name=f"qk{hf}t{t}") for t in range(8)]
              for hf in range(2)]
        # per-half v: [128 tokens, 8 token-tiles, HL heads, DH+1]
        vsb = [vpool.tile([128, 8, HL, DH + 1], BF16, tag=f"vsb{hf}",
                          name=f"vsb{hf}") for hf in range(2)]
        # per-q-half attention outputs: [128 (2 heads x 64), 1024 q-cols]
        ot = [[opool.tile([128, 1024], BF16, tag=f"ot{qh}_{k}",
                          name=f"ot{qh}_{k}") for k in range(4)]
              for qh in range(2)]

        xts = [None, None]

        def load_x(half):
            hs0 = half * 1024
            xth = []
            for k in range(KC):
                xh = xtp.tile([128, 1024], BF16, tag=f"xth{k}",
                              name=f"xth{k}")
                nc.sync.dma_start(xh[:], xT[ts(k, 128), ds(hs0, 1024)])
                xth.append(xh)
            xts[half] = xth
            if half == 0:
                # gate the const loads on the first x tile so their
                # transfers don't queue ahead of the critical-path
                # x/w tiles on the shared DMA engines; chunk to
                # <=512KB for fair interleaving
                gate = consts.tile([1, 8], BF16, tag="gate", name="gate")
                nc.gpsimd.tensor_copy(gate[:], xth[0][0:1, 0:8])
                # wv first (v-proj is the first consumer), wo last (not
                # needed until the output projection)
                for wh in range(2):
                    nc.gpsimd.dma_start(wv_sb[:, ts(wh, 4), :],
                                        wv[:, ts(wh, 4), :])
                for q2 in range(2):
                    nc.gpsimd.dma_start(cc_sb[:, ts(q2, 1024)],
                                        cc[:, ts(q2, 1024)])
                for q2 in range(2):
                    nc.gpsimd.dma_start(ssw_sb[:, ts(q2, 1024)],
                                        ssw[:, ts(q2, 1024)])
                nc.gpsimd.dma_start(perm_sb[:], perm)
                nc.gpsimd.dma_start(wo_sb[:], wo)
                nc.gpsimd.memset(vsb[0][:, :, :, DH], 1.0)
                nc.gpsimd.memset(vsb[1][:, :, :, DH], 1.0)

        def proj_v(half, pvb):
            # v projection, token tiles (half, 0..7)
            for t in range(8):
                psV = pvb.tile([128, 512], F32, tag="pvb")
                for k in range(KC):
                    nc.tensor.matmul(
                        psV[:], (xts[half][k][:, ds(t * 128, 128)]),
                        (wv_sb[:, k, :]),
                        start=(k == 0), stop=(k == KC - 1))
                nc.scalar.copy(
                    vsb[half][:, t, :, 0:DH],
                    psV[:].rearrange("p (h d) -> p h d", h=HL))

        def proj_qk(half, t, qsc, pvb):
            # q/k col-tile t of this half: project + rope, in two
            # 512-wide pieces (1 PSUM bank each)
            hs0 = half * 1024
            wt = wsl.tile([128, KC, 128], BF16, tag="w", name="w")
            nc.sync.dma_start(wt[:], wqk[t])
            for p in range(2):
                ps = qsc.tile([128, 512], F32, tag="qsc")
                for k in range(KC):
                    nc.tensor.matmul(
                        ps[:], (wt[:, k, :]),
                        (xts[half][k][:, ds(p * 512, 512)]),
                        start=(k == 0), stop=(k == KC - 1))
                csl = ds(hs0 + p * 512, 512)
                psb = rtmp.tile([128, 512], BF16, tag="psb")
                nc.scalar.copy(psb[:], ps[:])
                v2 = rtmp.tile([128, 512], BF16, tag="v2")
                nc.vector.tensor_mul(v2[:], psb[:], ssw_sb[:, csl])
                pb = pvb.tile([128, 512], F32, tag="pvb")
                nc.tensor.matmul(pb[:], perm_sb[:], v2[:],
                                 start=True, stop=True)
                t1 = rtmp.tile([128, 512], BF16, tag="t1")
                nc.vector.tensor_mul(t1[:], psb[:], cc_sb[:, csl])
                nc.vector.tensor_tensor(
                    qk[half][t][:, ds(p * 512, 512)], t1[:], pb[:],
                    op=mybir.AluOpType.add)

        def normalize(qh, ht, h, pav):
            # ot rows = pav[:64] / l, l = pav[64]
            lr = lpool.tile([128, 1024], F32, tag="lr")
            nc.vector.tensor_copy(lr[ds(64, 1), :], pav[ds(DH, 1), :])
            nc.sync.dma_start(lr[ds(0, 1), :], lr[ds(64, 1), :])
            # ~5x faster than InstReciprocal; ~51 ULP is plenty for the
            # softmax denominator.  Input/output stay on the proven SBUF
            # row-0 path (base-64 sources break on hardware).
            nc.vector.reciprocal_approx_fast(out=lr[ds(0, 1), :],
                                             in_=lr[ds(0, 1), :])
            rb = lpool.tile([64, 1024], F32, tag="rb")
            nc.gpsimd.partition_broadcast(rb[:], lr[ds(0, 1), :],
                                          channels=64)
            if h % 2 == 0:
                nc.vector.tensor_mul(
                    ot[qh][ht][ds(0, 64), :], pav[ds(0, DH), :], rb[:])
            else:
                ott = lpool.tile([64, 1024], BF16, tag="ott")
                nc.vector.tensor_mul(ott[:], pav[ds(0, DH), :], rb[:])
                nc.sync.dma_start(ot[qh][ht][ds(64, 64), :], ott[:])

        def av_mm(qh, h, j, pav, pj, gs):
            # attn @ v for block (h, j), accumulated over j into pav
            q0, nj = 1024 * qh, 8 * (qh + 1)
            vt = vsb[j // 8][:, j % 8, h, :]
            for c in range(max(2 * qh, j // 4), 2 * qh + 2):
                cs = max(512 * c, 128 * j)
                w = 512 * (c + 1) - cs
                nc.tensor.matmul(
                    pav[:, ds(cs - q0, w)], vt, (pj[:, ds(cs - gs, w)]),
                    start=(j == 0), stop=(j == min(nj - 1, 4 * c + 3)))

        def mask_diag(pj):
            # causal-mask the 128-col diagonal block at the tile start
            nc.gpsimd.affine_select(
                out=pj[:, 0:128], in_=pj[:, 0:128],
                compare_op=mybir.AluOpType.is_ge, fill=0.0,
                base=0, pattern=[[1, 128]],
                channel_multiplier=-1)

        def attn0(ht, qsc, psav):
            # first-q-half attention for head pair ht, one head at a
            # time.  Scores in 512-wide pieces sharing the proj psum tag
            # (4-deep ring keeps the exp pipeline 2+ tiles deep).
            for h in (2 * ht, 2 * ht + 1):
                hb = 64 * (h % 2)
                pav = psav.tile([DH + 1, 1024], F32, tag="pav",
                                name=f"pav{h}")
                for j in range(8):
                    gs = 128 * j
                    cw = 1024 - gs
                    pj = ppool.tile([128, cw], BF16, tag="P",
                                    name=f"pj{h}")
                    for (po, pw) in _pieces(cw):
                        ps = qsc.tile([128, 512], F32, tag="qsc",
                                      name=f"sc{h}")
                        nc.tensor.matmul(
                            ps[:, ds(0, pw)],
                            (qk[0][4 + ht][ds(hb, 64), ds(gs, 128)]),
                            (qk[0][ht][ds(hb, 64), ds(gs + po, pw)]),
                            start=True, stop=True)
                        nc.scalar.activation(pj[:, ds(po, pw)],
                                             ps[:, ds(0, pw)],
                                             EXP, scale=SCALE)
                        if po == 0:
                            mask_diag(pj)
                    av_mm(0, h, j, pav, pj, gs)
                normalize(0, ht, h, pav)

        def attn1(ht, pssc, psav):
            # second-q-half attention, one head at a time (ACT-bound
            # phase: full-width 1024-col exp calls minimize ACT overhead;
            # odd head first so its ot DMA overlaps the even head)
            for h in (2 * ht + 1, 2 * ht):
                hb = 64 * (h % 2)
                pav = psav.tile([DH + 1, 1024], F32, tag="pav")
                for j in range(16):
                    gs = max(1024, 128 * j)
                    cw = 2048 - gs
                    kt = qk[j // 8][4 + ht]
                    k_ap = kt[ds(hb, 64), ds(128 * (j % 8), 128)]
                    ps = pssc.tile([128, cw], F32, tag="sc")
                    for (po, pw) in _pieces(cw):
                        nc.tensor.matmul(
                            ps[:, ds(po, pw)], k_ap,
                            (qk[1][ht][ds(hb, 64), ds(gs - 1024 + po, pw)]),
                            start=True, stop=True)
                    pj = ppool.tile([128, cw], BF16, tag="P")
                    nc.scalar.activation(pj[:], ps[:], EXP, scale=SCALE)
                    if gs == 128 * j:
                        mask_diag(pj)
                    av_mm(1, h, j, pav, pj, gs)
                normalize(1, ht, h, pav)

        def outproj(tt, psy, tag="sc"):
            # y[tt*128 : tt*128+128, :] — needs ot[tt // 8][k] for all k.
            # PSUM comes out of the qh1 scores ring (same shape/tag) so
            # the attention pools keep full double-buffering; the tail
            # calls alternate into the freed attention-accumulator banks.
            # y is staged out in 512-wide pieces through a 4-deep ring so
            # the DMA latency never gates the matmuls.
            oth = ot[tt // 8]
            pp = psy.tile([128, 1024], F32, tag=tag, name=f"pp{tt}")
            for k in range(4):
                for half in range(2):
                    nc.tensor.matmul(
                        pp[:, ts(half, 512)], (oth[k][:, ts(tt % 8, 128)]),
                        (wo_sb[:, k, ts(half, 512)]),
                        start=(k == 0), stop=(k == 3))
            for half in range(2):
                yp = ypool.tile([128, 512], F32, tag="y")
                nc.vector.tensor_copy(yp[:], pp[:, ts(half, 512)])
                nc.sync.dma_start(y[ts(tt, 128), ds(half * 512, 512)],
                                  yp[:])

        # ---- stage 1: proj h0 -> attn qh0 || proj h1 ------------------
        with ExitStack() as st1:
            qsc = st1.enter_context(
                tc.tile_pool(name="qsc", bufs=4, space="PSUM"))
            pvb = st1.enter_context(
                tc.tile_pool(name="pvb", bufs=2, space="PSUM"))
            psav0 = st1.enter_context(
                tc.tile_pool(name="psav0", bufs=1, space="PSUM"))
            load_x(0)
            proj_v(0, pvb)
            proj_qk(0, 0, qsc, pvb)
            proj_qk(0, 4, qsc, pvb)
            attn0(0, qsc, psav0)
            load_x(1)
            proj_qk(0, 1, qsc, pvb)
            proj_qk(0, 5, qsc, pvb)
            proj_v(1, pvb)
            attn0(1, qsc, psav0)
            for ht in (2, 3):
                proj_qk(0, ht, qsc, pvb)
                proj_qk(0, 4 + ht, qsc, pvb)
                proj_qk(1, ht - 2, qsc, pvb)
                proj_qk(1, ht + 2, qsc, pvb)
                attn0(ht, qsc, psav0)
            for t in (2, 6, 3, 7):
                proj_qk(1, t, qsc, pvb)

        # ---- stage 2: attn qh1 || outproj -----------------------------
        with ExitStack() as st2:
            pss1 = st2.enter_context(
                tc.tile_pool(name="pss1", bufs=2, space="PSUM"))
            psav1 = st2.enter_context(
                tc.tile_pool(name="psav1", bufs=2, space="PSUM"))
            for ht in range(4):
                attn1(ht, pss1, psav1)
                outproj(2 * ht, pss1)
                outproj(2 * ht + 1, pss1)
            for i, tt in enumerate(range(8, 16)):
                if i % 2 == 0:
                    outproj(tt, pss1, tag="sc")
                else:
                    outproj(tt, psav1, tag="pav")
    return nc


# ---------------- host side ------------------------------------------------

def _bf16(a):
    import ml_dtypes
    return np.ascontiguousarray(np.asarray(a).astype(ml_dtypes.bfloat16))


def _rope_tables():
    i = np.arange(DH // 2, dtype=np.float32)
    thetas = np.power(np.float32(10000.0), -2.0 * (i - 1.0) / DH)
    vals = thetas[:, None].astype(np.float32) * \
        np.arange(S, dtype=np.float32)[None, :]
    cos32 = np.cos(vals).astype(np.float32)
    sin32 = np.sin(vals).astype(np.float32)
    CC = np.tile(cos32, (4, 1))
    SSsw = np.concatenate([sin32, -sin32, sin32, -sin32], axis=0)
    return _bf16(CC), _bf16(SSsw)


def _perm_matrix():
    P = np.zeros((128, 128), dtype=np.float32)
    for m in range(128):
        P[m ^ 32, m] = 1.0
    return P


def _qk_col_perm(g):
    cols = []
    for m in range(4):
        for hh in (2 * m, 2 * m + 1):
            hg = HL * g + hh
            cols += [hg * DH + 2 * i for i in range(32)]
            cols += [hg * DH + 2 * i + 1 for i in range(32)]
    return np.array(cols)


_CACHE = {}


def _get_module():
    if "nc" not in _CACHE:
        nc = bacc.Bacc("TRN2", target_bir_lowering=False, debug=False,
                       num_devices=8)
        build_kernel(nc)
        nc.compile()
        _CACHE["nc"] = nc
    return _CACHE["nc"]


def make_in_maps(x, Wqkv, Wout):
    x = np.ascontiguousarray(np.asarray(x, np.float32))
    Wqkv = np.ascontiguousarray(np.asarray(Wqkv, np.float32))
    Wout = np.ascontiguousarray(np.asarray(Wout, np.float32))
    CC, SSsw = _rope_tables()
    P = _bf16(_perm_matrix())
    shard = {}
    for g in range(2):
        perm = _qk_col_perm(g)
        vcols = np.arange(HL * g * DH, HL * (g + 1) * DH)
        wqp = Wqkv[:, 0 * INNER:1 * INNER][:, perm]
        wkp = Wqkv[:, 1 * INNER:2 * INNER][:, perm]
        # [t, p, k, n]: tile t's [D, 128] block split into 8 [128, 128]
        # contraction chunks, partition-major
        wqk = np.stack([
            (wqp if t < 4 else wkp)[:, (t % 4) * 128:(t % 4 + 1) * 128]
            .reshape(KC, 128, 128).transpose(1, 0, 2)
            for t in range(8)])
        shard[g] = dict(
            wqk=_bf16(wqk),
            wv=_bf16(Wqkv[:, 2 * INNER:3 * INNER][:, vcols]
                     .reshape(KC, 128, 512).transpose(1, 0, 2)),
            wo=_bf16(Wout[vcols, :].reshape(4, 128, D).transpose(1, 0, 2)),
        )
    in_maps = []
    for c in range(8):
        b, g = c // 2, c % 2
        in_maps.append(dict(
            xT=_bf16(x[b].T), cc=CC, ssw=SSsw, perm=P, **shard[g]))
    return in_maps


def kernel(x, Wqkv, Wout, bout):
    bout = np.asarray(bout, np.float32)
    nc = _get_module()
    in_maps = make_in_maps(x, Wqkv, Wout)
    res = run_bass_kernel_spmd(nc, in_maps, core_ids=list(range(8)))
    ys = [r["y"] for r in res.results]
    out = np.stack([ys[2 * b] + ys[2 * b + 1] + bout for b in range(B)])
    return out.astype(np.float32)


# revision 22
# speedup vs baseline: 1.0185x; 1.0185x over previous
"""Trainium2 Bass kernel for nn_Attention (dense transformer block:
qkv proj + RoPE + causal attention + out proj), tensor-parallel over
8 NeuronCores: core c handles batch b=c//2, head-group g=c%2 (8 heads).

Self-contained: hardcodes all shapes; host preps transposed/permuted
shards, device computes partial y per core, host sums head-group pairs
and adds the output bias.

All matmul operands are bf16 (1 cyc/row at any width, half the HBM
traffic; accumulation stays fp32 in PSUM, softmax denominators and the
final output stay fp32).  The RoPE partition swap is a permutation
matmul on the tensor engine.  Weights are pre-arranged on the host so
every weight load is one large contiguous DMA.

This version overlaps the three phases: all SBUF tiles are split by
sequence half (qk0/qk1, vsb0/vsb1) and by q-half (ot0/ot1) so the Tile
scheduler's dependency tracking lets attention on the first 1024
q-columns run concurrently with the second half's projections, and the
output projection of the first q-half runs inside the (ACT-bound)
second-half attention.  The scalar engine does exp only; all PSUM
evictions run on the vector engine.  PSUM is replanned so each set of
concurrently-live pools fits in 8 banks.
"""

from contextlib import ExitStack

import numpy as np

import concourse.bass as bass
import concourse.tile as tile
from concourse import bacc, mybir
from concourse.bass import ds, ts
from concourse.bass_utils import run_bass_kernel_spmd

B, S, D, H, DH = 4, 2048, 1024, 16, 64
HL = 8          # heads per core
INNER = H * DH  # 1024
KC = D // 128   # 8 contraction chunks
NT = S // 128   # 16 token tiles
F32 = mybir.dt.float32
BF16 = mybir.dt.bfloat16


def _pieces(cw):
    """split a psum-tile column span into single-bank matmul pieces"""
    out = [(i * 512, 512) for i in range(cw // 512)]
    if cw % 512:
        out.append((cw - cw % 512, cw % 512))
    return out


def build_kernel(nc, phases=3, loop_n=0):
    xT = nc.dram_tensor("xT", [D, S], BF16, kind="ExternalInput").ap()
    # [t, p, k, n]: per q/k col-tile t, contraction chunk k pre-split
    wqk = nc.dram_tensor("wqk", [8, 128, KC, 128], BF16,
                         kind="ExternalInput").ap()
    wv = nc.dram_tensor("wv", [128, KC, 512], BF16, kind="ExternalInput").ap()
    wo = nc.dram_tensor("wo", [128, 4, D], BF16, kind="ExternalInput").ap()
    cc = nc.dram_tensor("cc", [128, S], BF16, kind="ExternalInput").ap()
    ssw = nc.dram_tensor("ssw", [128, S], BF16, kind="ExternalInput").ap()
    perm = nc.dram_tensor("perm", [128, 128], BF16, kind="ExternalInput").ap()
    y = nc.dram_tensor("y", [S, D], F32, kind="ExternalOutput").ap()

    EXP = mybir.ActivationFunctionType.Exp
    SCALE = 1.0 / np.sqrt(DH)

    with tile.TileContext(nc) as tc, ExitStack() as top:
        if loop_n:
            top.enter_context(tc.For_i(0, loop_n, 1,
                                       hint_engines=(mybir.EngineType.PE,)))
        # ---- long-lived SBUF pools -----------------------------------
        consts = top.enter_context(tc.tile_pool(name="consts", bufs=1))
        wop = top.enter_context(tc.tile_pool(name="wop", bufs=1))
        xtp = top.enter_context(tc.tile_pool(name="xtp", bufs=2))
        wsl = top.enter_context(tc.tile_pool(name="wsl", bufs=2))
        rtmp = top.enter_context(tc.tile_pool(name="rtmp", bufs=3))
        qkp = top.enter_context(tc.tile_pool(name="qkt", bufs=1))
        vpool = top.enter_context(tc.tile_pool(name="vpool", bufs=1))
        opool = top.enter_context(tc.tile_pool(name="opool", bufs=1))
        ppool = top.enter_context(tc.tile_pool(name="ppool", bufs=8))
        lpool = top.enter_context(tc.tile_pool(name="lpool", bufs=2))
        ypool = top.enter_context(tc.tile_pool(name="ypool", bufs=3))

        cc_sb = consts.tile([128, S], BF16, tag="cc", name="cc")
        ssw_sb = consts.tile([128, S], BF16, tag="ssw", name="ssw")
        perm_sb = consts.tile([128, 128], BF16, tag="perm", name="perm")
        wv_sb = consts.tile([128, KC, 512], BF16, tag="wv", name="wv")
        wo_sb = wop.tile([128, 4, D], BF16, tag="wo", name="wo")

        # per-half q/k tiles: [128 (2 heads x 64 rope dims), 1024 tokens]
        qk = [[qkp.tile([128, 1024], BF16, tag=f"qk{hf}t{t}",
                        name=f"qk{hf}t{t}") for t in range(8)]
              for hf in range(2)]
        # per-half v: [128 tokens, 8 token-tiles, HL heads, DH+1]
        vsb = [vpool.tile([128, 8, HL, DH + 1], BF16, tag=f"vsb{hf}",
                          name=f"vsb{hf}") for hf in range(2)]
        # per-q-half attention outputs: [128 (2 heads x 64), 1024 q-cols]
        ot = [[opool.tile([128, 1024], BF16, tag=f"ot{qh}_{k}",
                          name=f"ot{qh}_{k}") for k in range(4)]
              for qh in range(2)]

        xts = [None, None]

        def load_x(half):
            hs0 = half * 1024
            xth = []
            for k in range(KC):
                xh = xtp.tile([128, 1024], BF16, tag=f"xth{k}",
                              name=f"xth{k}")
                nc.sync.dma_start(xh[:], xT[ts(k, 128), ds(hs0, 1024)])
                xth.append(xh)
            xts[half] = xth
            if half == 0:
                # gate the const loads on the first x tile so their
                # transfers don't queue ahead of the critical-path
                # x/w tiles on the shared DMA engines; chunk to
                # <=512KB for fair interleaving
                gate = consts.tile([1, 8], BF16, tag="gate", name="gate")
                nc.gpsimd.tensor_copy(gate[:], xth[0][0:1, 0:8])
                # wv first (v-proj is the first consumer), wo last (not
                # needed until the output projection)
                for wh in range(2):
                    nc.gpsimd.dma_start(wv_sb[:, ts(wh, 4), :],
                                        wv[:, ts(wh, 4), :])
                for q2 in range(2):
                    nc.gpsimd.dma_start(cc_sb[:, ts(q2, 1024)],
                                        cc[:, ts(q2, 1024)])
                for q2 in range(2):
                    nc.gpsimd.dma_start(ssw_sb[:, ts(q2, 1024)],
                                        ssw[:, ts(q2, 1024)])
                nc.gpsimd.dma_start(perm_sb[:], perm)
                nc.gpsimd.dma_start(wo_sb[:], wo)
                nc.gpsimd.memset(vsb[0][:, :, :, DH], 1.0)
                nc.gpsimd.memset(vsb[1][:, :, :, DH], 1.0)

        def proj_v(half, pvb):
            # v projection, token tiles (half, 0..7)
            for t in range(8):
                psV = pvb.tile([128, 512], F32, tag="pvb")
                for k in range(KC):
                    nc.tensor.matmul(
                        psV[:], (xts[half][k][:, ds(t * 128, 128)]),
                        (wv_sb[:, k, :]),
                        start=(k == 0), stop=(k == KC - 1))
                nc.scalar.copy(
                    vsb[half][:, t, :, 0:DH],
                    psV[:].rearrange("p (h d) -> p h d", h=HL))

        def proj_qk(half, t, qsc, pvb):
            # q/k col-tile t of this half: project + rope, in two
            # 512-wide pieces (1 PSUM bank each)
            hs0 = half * 1024
            wt = wsl.tile([128, KC, 128], BF16, tag="w", name="w")
            nc.sync.dma_start(wt[:], wqk[t])
            for p in range(2):
                ps = qsc.tile([128, 512], F32, tag="qsc")
                for k in range(KC):
                    nc.tensor.matmul(
                        ps[:], (wt[:, k, :]),
                        (xts[half][k][:, ds(p * 512, 512)]),
                        start=(k == 0), stop=(k == KC - 1))
                csl = ds(hs0 + p * 512, 512)
                psb = rtmp.tile([128, 512], BF16, tag="psb")
                nc.scalar.copy(psb[:], ps[:])
                v2 = rtmp.tile([128, 512], BF16, tag="v2")
                nc.vector.tensor_mul(v2[:], psb[:], ssw_sb[:, csl])
                pb = pvb.tile([128, 512], F32, tag="pvb")
                nc.tensor.matmul(pb[:], perm_sb[:], v2[:],
                                 start=True, stop=True)
                t1 = rtmp.tile([128, 512], BF16, tag="t1")
                nc.vector.tensor_mul(t1[:], psb[:], cc_sb[:, csl])
                nc.vector.tensor_tensor(
                    qk[half][t][:, ds(p * 512, 512)], t1[:], pb[:],
                    op=mybir.AluOpType.add)

        def normalize(qh, ht, h, pav):
            # ot rows = pav[:64] / l, l = pav[64]
            lr = lpool.tile([128, 1024], F32, tag="lr")
            nc.vector.tensor_copy(lr[ds(64, 1), :], pav[ds(DH, 1), :])
            nc.sync.dma_start(lr[ds(0, 1), :], lr[ds(64, 1), :])
            # ~5x faster than InstReciprocal; ~51 ULP is plenty for the
            # softmax denominator.  Input/output stay on the proven SBUF
            # row-0 path (base-64 sources break on hardware).
            nc.vector.reciprocal_approx_fast(out=lr[ds(0, 1), :],
                                             in_=lr[ds(0, 1), :])
            rb = lpool.tile([64, 1024], F32, tag="rb")
            nc.gpsimd.partition_broadcast(rb[:], lr[ds(0, 1), :],
                                          channels=64)
            if h % 2 == 0:
                nc.vector.tensor_mul(
                    ot[qh][ht][ds(0, 64), :], pav[ds(0, DH), :], rb[:])
            else:
                ott = lpool.tile([64, 1024], BF16, tag="ott")
                nc.vector.tensor_mul(ott[:], pav[ds(0, DH), :], rb[:])
                nc.sync.dma_start(ot[qh][ht][ds(64, 64), :], ott[:])

        def av_mm(qh, h, j, pav, pj, gs):
            # attn @ v for block (h, j), accumulated over j into pav
            q0, nj = 1024 * qh, 8 * (qh + 1)
            vt = vsb[j // 8][:, j % 8, h, :]
            for c in range(max(2 * qh, j // 4), 2 * qh + 2):
                cs = max(512 * c, 128 * j)
                w = 512 * (c + 1) - cs
                nc.tensor.matmul(
                    pav[:, ds(cs - q0, w)], vt, (pj[:, ds(cs - gs, w)]),
                    start=(j == 0), stop=(j == min(nj - 1, 4 * c + 3)))

        def mask_diag(pj):
            # causal-mask the 128-col diagonal block at the tile start
            nc.gpsimd.affine_select(
                out=pj[:, 0:128], in_=pj[:, 0:128],
                compare_op=mybir.AluOpType.is_ge, fill=0.0,
                base=0, pattern=[[1, 128]],
                channel_multiplier=-1)

        def attn0(ht, qsc, psav):
            # first-q-half attention for head pair ht, one head at a
            # time.  Scores in 512-wide pieces sharing the proj psum tag
            # (4-deep ring keeps the exp pipeline 2+ tiles deep).
            for h in (2 * ht, 2 * ht + 1):
                hb = 64 * (h % 2)
                pav = psav.tile([DH + 1, 1024], F32, tag="pav",
                                name=f"pav{h}")
                for j in range(8):
                    gs = 128 * j
                    cw = 1024 - gs
                    pj = ppool.tile([128, cw], BF16, tag="P",
                                    name=f"pj{h}")
                    for (po, pw) in _pieces(cw):
                        ps = qsc.tile([128, 512], F32, tag="qsc",
                                      name=f"sc{h}")
                        nc.tensor.matmul(
                            ps[:, ds(0, pw)],
                            (qk[0][4 + ht][ds(hb, 64), ds(gs, 128)]),
                            (qk[0][ht][ds(hb, 64), ds(gs + po, pw)]),
                            start=True, stop=True)
                        nc.scalar.activation(pj[:, ds(po, pw)],
                                             ps[:, ds(0, pw)],
                                             EXP, scale=SCALE)
                        if po == 0:
                            mask_diag(pj)
                    av_mm(0, h, j, pav, pj, gs)
                normalize(0, ht, h, pav)

        def attn1(ht, pssc, psav):
            # second-q-half attention, one head at a time (ACT-bound
            # phase: full-width 1024-col exp calls minimize ACT overhead;
            # odd head first so its ot DMA overlaps the even head)
            for h in (2 * ht + 1, 2 * ht):
                hb = 64 * (h % 2)
                pav = psav.tile([DH + 1, 1024], F32, tag="pav")
                for j in range(16):
                    gs = max(1024, 128 * j)
                    cw = 2048 - gs
                    kt = qk[j // 8][4 + ht]
                    k_ap = kt[ds(hb, 64), ds(128 * (j % 8), 128)]
                    ps = pssc.tile([128, cw], F32, tag="sc")
                    for (po, pw) in _pieces(cw):
                        nc.tensor.matmul(
                            ps[:, ds(po, pw)], k_ap,
                            (qk[1][ht][ds(hb, 64), ds(gs - 1024 + po, pw)]),
                            start=True, stop=True)
                    pj = ppool.tile([128, cw], BF16, tag="P")
                    nc.scalar.activation(pj[:], ps[:], EXP, scale=SCALE)
                    if gs == 128 * j:
                        mask_diag(pj)
                    av_mm(1, h, j, pav, pj, gs)
                normalize(1, ht, h, pav)

        def outproj(tt, psy, tag="sc"):
            # y[tt*128 : tt*128+128, :] — needs ot[tt // 8][k] for all k.
            # PSUM comes out of the qh1 scores ring (same shape/tag) so
            # the attention pools keep full double-buffering; tail calls
            # alternate into the freed attention-accumulator banks so four
            # PSUM tiles rotate there.  y stays one full-width DMA per
            # tile (32 piece-DMAs measured slower on HW) with a 3-deep
            # staging ring so DMA latency does not gate the matmuls.
            oth = ot[tt // 8]
            ysb = ypool.tile([128, D], F32, tag="y")
            pp = psy.tile([128, 1024], F32, tag=tag, name=f"pp{tt}")
            for k in range(4):
                for half in range(2):
                    nc.tensor.matmul(
                        pp[:, ts(half, 512)], (oth[k][:, ts(tt % 8, 128)]),
                        (wo_sb[:, k, ts(half, 512)]),
                        start=(k == 0), stop=(k == 3))
            nc.vector.tensor_copy(ysb[:], pp[:])
            nc.sync.dma_start(y[ts(tt, 128), :], ysb[:])

        # ---- stage 1: proj h0 -> attn qh0 || proj h1 ------------------
        with ExitStack() as st1:
            qsc = st1.enter_context(
                tc.tile_pool(name="qsc", bufs=4, space="PSUM"))
            pvb = st1.enter_context(
                tc.tile_pool(name="pvb", bufs=2, space="PSUM"))
            psav0 = st1.enter_context(
                tc.tile_pool(name="psav0", bufs=1, space="PSUM"))
            load_x(0)
            proj_v(0, pvb)
            proj_qk(0, 0, qsc, pvb)
            proj_qk(0, 4, qsc, pvb)
            attn0(0, qsc, psav0)
            load_x(1)
            proj_qk(0, 1, qsc, pvb)
            proj_qk(0, 5, qsc, pvb)
            proj_v(1, pvb)
            attn0(1, qsc, psav0)
            for ht in (2, 3):
                proj_qk(0, ht, qsc, pvb)
                proj_qk(0, 4 + ht, qsc, pvb)
                proj_qk(1, ht - 2, qsc, pvb)
                proj_qk(1, ht + 2, qsc, pvb)
                attn0(ht, qsc, psav0)
            for t in (2, 6, 3, 7):
                proj_qk(1, t, qsc, pvb)

        # ---- stage 2: attn qh1 || outproj -----------------------------
        with ExitStack() as st2:
            pss1 = st2.enter_context(
                tc.tile_pool(name="pss1", bufs=2, space="PSUM"))
            psav1 = st2.enter_context(
                tc.tile_pool(name="psav1", bufs=2, space="PSUM"))
            for ht in range(4):
                attn1(ht, pss1, psav1)
                outproj(2 * ht, pss1)
                outproj(2 * ht + 1, pss1)
            for i, tt in enumerate(range(8, 16)):
                if i % 2 == 0:
                    outproj(tt, pss1, tag="sc")
                else:
                    outproj(tt, psav1, tag="pav")
    return nc


# ---------------- host side ------------------------------------------------

def _bf16(a):
    import ml_dtypes
    return np.ascontiguousarray(np.asarray(a).astype(ml_dtypes.bfloat16))


def _rope_tables():
    i = np.arange(DH // 2, dtype=np.float32)
    thetas = np.power(np.float32(10000.0), -2.0 * (i - 1.0) / DH)
    vals = thetas[:, None].astype(np.float32) * \
        np.arange(S, dtype=np.float32)[None, :]
    cos32 = np.cos(vals).astype(np.float32)
    sin32 = np.sin(vals).astype(np.float32)
    CC = np.tile(cos32, (4, 1))
    SSsw = np.concatenate([sin32, -sin32, sin32, -sin32], axis=0)
    return _bf16(CC), _bf16(SSsw)


def _perm_matrix():
    P = np.zeros((128, 128), dtype=np.float32)
    for m in range(128):
        P[m ^ 32, m] = 1.0
    return P


def _qk_col_perm(g):
    cols = []
    for m in range(4):
        for hh in (2 * m, 2 * m + 1):
            hg = HL * g + hh
            cols += [hg * DH + 2 * i for i in range(32)]
            cols += [hg * DH + 2 * i + 1 for i in range(32)]
    return np.array(cols)


_CACHE = {}


def _get_module():
    if "nc" not in _CACHE:
        nc = bacc.Bacc("TRN2", target_bir_lowering=False, debug=False,
                       num_devices=8)
        build_kernel(nc)
        nc.compile()
        _CACHE["nc"] = nc
    return _CACHE["nc"]


def make_in_maps(x, Wqkv, Wout):
    x = np.ascontiguousarray(np.asarray(x, np.float32))
    Wqkv = np.ascontiguousarray(np.asarray(Wqkv, np.float32))
    Wout = np.ascontiguousarray(np.asarray(Wout, np.float32))
    CC, SSsw = _rope_tables()
    P = _bf16(_perm_matrix())
    shard = {}
    for g in range(2):
        perm = _qk_col_perm(g)
        vcols = np.arange(HL * g * DH, HL * (g + 1) * DH)
        wqp = Wqkv[:, 0 * INNER:1 * INNER][:, perm]
        wkp = Wqkv[:, 1 * INNER:2 * INNER][:, perm]
        # [t, p, k, n]: tile t's [D, 128] block split into 8 [128, 128]
        # contraction chunks, partition-major
        wqk = np.stack([
            (wqp if t < 4 else wkp)[:, (t % 4) * 128:(t % 4 + 1) * 128]
            .reshape(KC, 128, 128).transpose(1, 0, 2)
            for t in range(8)])
        shard[g] = dict(
            wqk=_bf16(wqk),
            wv=_bf16(Wqkv[:, 2 * INNER:3 * INNER][:, vcols]
                     .reshape(KC, 128, 512).transpose(1, 0, 2)),
            wo=_bf16(Wout[vcols, :].reshape(4, 128, D).transpose(1, 0, 2)),
        )
    in_maps = []
    for c in range(8):
        b, g = c // 2, c % 2
        in_maps.append(dict(
            xT=_bf16(x[b].T), cc=CC, ssw=SSsw, perm=P, **shard[g]))
    return in_maps


def kernel(x, Wqkv, Wout, bout):
    bout = np.asarray(bout, np.float32)
    nc = _get_module()
    in_maps = make_in_maps(x, Wqkv, Wout)
    res = run_bass_kernel_spmd(nc, in_maps, core_ids=list(range(8)))
    ys = [r["y"] for r in res.results]
    out = np.stack([ys[2 * b] + ys[2 * b + 1] + bout for b in range(B)])
    return out.astype(np.float32)


# revision 26
# speedup vs baseline: 1.0247x; 1.0060x over previous
"""Trainium2 Bass kernel for nn_Attention (dense transformer block:
qkv proj + RoPE + causal attention + out proj), tensor-parallel over
8 NeuronCores: core c handles batch b=c//2, head-group g=c%2 (8 heads).

Self-contained: hardcodes all shapes; host preps transposed/permuted
shards, device computes partial y per core, host sums head-group pairs
and adds the output bias.

All matmul operands are bf16 (1 cyc/row at any width, half the HBM
traffic; accumulation stays fp32 in PSUM, softmax denominators and the
final output stay fp32).  The RoPE partition swap is a permutation
matmul on the tensor engine.  Weights are pre-arranged on the host so
every weight load is one large contiguous DMA.

This version overlaps the three phases: all SBUF tiles are split by
sequence half (qk0/qk1, vsb0/vsb1) and by q-half (ot0/ot1) so the Tile
scheduler's dependency tracking lets attention on the first 1024
q-columns run concurrently with the second half's projections, and the
output projection of the first q-half runs inside the (ACT-bound)
second-half attention.  The scalar engine does exp only; all PSUM
evictions run on the vector engine.  PSUM is replanned so each set of
concurrently-live pools fits in 8 banks.
"""

from contextlib import ExitStack

import numpy as np

import concourse.bass as bass
import concourse.tile as tile
from concourse import bacc, mybir
from concourse.bass import ds, ts
from concourse.bass_utils import run_bass_kernel_spmd

B, S, D, H, DH = 4, 2048, 1024, 16, 64
HL = 8          # heads per core
INNER = H * DH  # 1024
KC = D // 128   # 8 contraction chunks
NT = S // 128   # 16 token tiles
F32 = mybir.dt.float32
BF16 = mybir.dt.bfloat16


def _pieces(cw):
    """split a psum-tile column span into single-bank matmul pieces"""
    out = [(i * 512, 512) for i in range(cw // 512)]
    if cw % 512:
        out.append((cw - cw % 512, cw % 512))
    return out


def build_kernel(nc, phases=3, loop_n=0):
    xT = nc.dram_tensor("xT", [D, S], BF16, kind="ExternalInput").ap()
    # [t, p, k, n]: per q/k col-tile t, contraction chunk k pre-split
    wqk = nc.dram_tensor("wqk", [8, 128, KC, 128], BF16,
                         kind="ExternalInput").ap()
    wv = nc.dram_tensor("wv", [128, KC, 512], BF16, kind="ExternalInput").ap()
    wo = nc.dram_tensor("wo", [128, 4, D], BF16, kind="ExternalInput").ap()
    cc = nc.dram_tensor("cc", [128, S], BF16, kind="ExternalInput").ap()
    ssw = nc.dram_tensor("ssw", [128, S], BF16, kind="ExternalInput").ap()
    perm = nc.dram_tensor("perm", [128, 128], BF16, kind="ExternalInput").ap()
    y = nc.dram_tensor("y", [S, D], F32, kind="ExternalOutput").ap()

    EXP = mybir.ActivationFunctionType.Exp
    SCALE = 1.0 / np.sqrt(DH)

    with tile.TileContext(nc) as tc, ExitStack() as top:
        if loop_n:
            top.enter_context(tc.For_i(0, loop_n, 1,
                                       hint_engines=(mybir.EngineType.PE,)))
        # ---- long-lived SBUF pools -----------------------------------
        consts = top.enter_context(tc.tile_pool(name="consts", bufs=1))
        wop = top.enter_context(tc.tile_pool(name="wop", bufs=1))
        xtp = top.enter_context(tc.tile_pool(name="xtp", bufs=2))
        wsl = top.enter_context(tc.tile_pool(name="wsl", bufs=2))
        rtmp = top.enter_context(tc.tile_pool(name="rtmp", bufs=3))
        qkp = top.enter_context(tc.tile_pool(name="qkt", bufs=1))
        vpool = top.enter_context(tc.tile_pool(name="vpool", bufs=1))
        opool = top.enter_context(tc.tile_pool(name="opool", bufs=1))
        ppool = top.enter_context(tc.tile_pool(name="ppool", bufs=10))
        lpool = top.enter_context(tc.tile_pool(name="lpool", bufs=2))
        ypool = top.enter_context(tc.tile_pool(name="ypool", bufs=2))

        cc_sb = consts.tile([128, S], BF16, tag="cc", name="cc")
        ssw_sb = consts.tile([128, S], BF16, tag="ssw", name="ssw")
        perm_sb = consts.tile([128, 128], BF16, tag="perm", name="perm")
        wv_sb = consts.tile([128, KC, 512], BF16, tag="wv", name="wv")
        wo_sb = wop.tile([128, 4, D], BF16, tag="wo", name="wo")

        # per-half q/k tiles: [128 (2 heads x 64 rope dims), 1024 tokens]
        qk = [[qkp.tile([128, 1024], BF16, tag=f"qk{hf}t{t}",
                        name=f"qk{hf}t{t}") for t in range(8)]
              for hf in range(2)]
        # per-half v: [128 tokens, 8 token-tiles, HL heads, DH+1]
        vsb = [vpool.tile([128, 8, HL, DH + 1], BF16, tag=f"vsb{hf}",
                          name=f"vsb{hf}") for hf in range(2)]
        # per-q-half attention outputs: [128 (2 heads x 64), 1024 q-cols]
        ot = [[opool.tile([128, 1024], BF16, tag=f"ot{qh}_{k}",
                          name=f"ot{qh}_{k}") for k in range(4)]
              for qh in range(2)]

        xts = [None, None]

        def load_x(half):
            hs0 = half * 1024
            xth = []
            for k in range(KC):
                xh = xtp.tile([128, 1024], BF16, tag=f"xth{k}",
                              name=f"xth{k}")
                nc.sync.dma_start(xh[:], xT[ts(k, 128), ds(hs0, 1024)])
                xth.append(xh)
            xts[half] = xth
            if half == 0:
                # gate the const loads on the first x tile so their
                # transfers don't queue ahead of the critical-path
                # x/w tiles on the shared DMA engines; chunk to
                # <=512KB for fair interleaving
                gate = consts.tile([1, 8], BF16, tag="gate", name="gate")
                nc.gpsimd.tensor_copy(gate[:], xth[0][0:1, 0:8])
                # wv first (v-proj is the first consumer), wo last (not
                # needed until the output projection)
                for wh in range(2):
                    nc.gpsimd.dma_start(wv_sb[:, ts(wh, 4), :],
                                        wv[:, ts(wh, 4), :])
                for q2 in range(2):
                    nc.gpsimd.dma_start(cc_sb[:, ts(q2, 1024)],
                                        cc[:, ts(q2, 1024)])
                for q2 in range(2):
                    nc.gpsimd.dma_start(ssw_sb[:, ts(q2, 1024)],
                                        ssw[:, ts(q2, 1024)])
                nc.gpsimd.dma_start(perm_sb[:], perm)
                nc.gpsimd.dma_start(wo_sb[:], wo)
                nc.gpsimd.memset(vsb[0][:, :, :, DH], 1.0)
                nc.gpsimd.memset(vsb[1][:, :, :, DH], 1.0)

        def proj_v(half, pvb):
            # v projection, token tiles (half, 0..7)
            for t in range(8):
                psV = pvb.tile([128, 512], F32, tag="pvb")
                for k in range(KC):
                    nc.tensor.matmul(
                        psV[:], (xts[half][k][:, ds(t * 128, 128)]),
                        (wv_sb[:, k, :]),
                        start=(k == 0), stop=(k == KC - 1))
                nc.scalar.copy(
                    vsb[half][:, t, :, 0:DH],
                    psV[:].rearrange("p (h d) -> p h d", h=HL))

        def proj_qk(half, t, qsc, pvb):
            # q/k col-tile t of this half: project + rope, in two
            # 512-wide pieces (1 PSUM bank each)
            hs0 = half * 1024
            wt = wsl.tile([128, KC, 128], BF16, tag="w", name="w")
            nc.sync.dma_start(wt[:], wqk[t])
            for p in range(2):
                ps = qsc.tile([128, 512], F32, tag="qsc")
                for k in range(KC):
                    nc.tensor.matmul(
                        ps[:], (wt[:, k, :]),
                        (xts[half][k][:, ds(p * 512, 512)]),
                        start=(k == 0), stop=(k == KC - 1))
                csl = ds(hs0 + p * 512, 512)
                psb = rtmp.tile([128, 512], BF16, tag="psb")
                nc.scalar.copy(psb[:], ps[:])
                v2 = rtmp.tile([128, 512], BF16, tag="v2")
                nc.vector.tensor_mul(v2[:], psb[:], ssw_sb[:, csl])
                pb = pvb.tile([128, 512], F32, tag="pvb")
                nc.tensor.matmul(pb[:], perm_sb[:], v2[:],
                                 start=True, stop=True)
                t1 = rtmp.tile([128, 512], BF16, tag="t1")
                nc.vector.tensor_mul(t1[:], psb[:], cc_sb[:, csl])
                nc.vector.tensor_tensor(
                    qk[half][t][:, ds(p * 512, 512)], t1[:], pb[:],
                    op=mybir.AluOpType.add)

        def normalize(qh, ht, h, pav):
            # ot rows = pav[:64] / l, l = pav[64]
            lr = lpool.tile([128, 1024], F32, tag="lr")
            nc.vector.tensor_copy(lr[ds(64, 1), :], pav[ds(DH, 1), :])
            nc.sync.dma_start(lr[ds(0, 1), :], lr[ds(64, 1), :])
            # ~5x faster than InstReciprocal; ~51 ULP is plenty for the
            # softmax denominator.  Input/output stay on the proven SBUF
            # row-0 path (base-64 sources break on hardware).
            nc.vector.reciprocal_approx_fast(out=lr[ds(0, 1), :],
                                             in_=lr[ds(0, 1), :])
            rb = lpool.tile([64, 1024], F32, tag="rb")
            nc.gpsimd.partition_broadcast(rb[:], lr[ds(0, 1), :],
                                          channels=64)
            if h % 2 == 0:
                nc.vector.tensor_mul(
                    ot[qh][ht][ds(0, 64), :], pav[ds(0, DH), :], rb[:])
            else:
                ott = lpool.tile([64, 1024], BF16, tag="ott")
                nc.vector.tensor_mul(ott[:], pav[ds(0, DH), :], rb[:])
                nc.sync.dma_start(ot[qh][ht][ds(64, 64), :], ott[:])

        def av_mm(qh, h, j, pav, pj, gs):
            # attn @ v for block (h, j), accumulated over j into pav
            q0, nj = 1024 * qh, 8 * (qh + 1)
            vt = vsb[j // 8][:, j % 8, h, :]
            for c in range(max(2 * qh, j // 4), 2 * qh + 2):
                cs = max(512 * c, 128 * j)
                w = 512 * (c + 1) - cs
                nc.tensor.matmul(
                    pav[:, ds(cs - q0, w)], vt, (pj[:, ds(cs - gs, w)]),
                    start=(j == 0), stop=(j == min(nj - 1, 4 * c + 3)))

        def mask_diag(pj):
            # causal-mask the 128-col diagonal block at the tile start
            nc.gpsimd.affine_select(
                out=pj[:, 0:128], in_=pj[:, 0:128],
                compare_op=mybir.AluOpType.is_ge, fill=0.0,
                base=0, pattern=[[1, 128]],
                channel_multiplier=-1)

        def attn0(ht, qsc, psav):
            # first-q-half attention for head pair ht, one head at a
            # time.  Scores in 512-wide pieces sharing the proj psum tag
            # (4-deep ring keeps the exp pipeline 2+ tiles deep).
            for h in (2 * ht, 2 * ht + 1):
                hb = 64 * (h % 2)
                pav = psav.tile([DH + 1, 1024], F32, tag="pav",
                                name=f"pav{h}")
                prev = None
                for j in range(8):
                    gs = 128 * j
                    cw = 1024 - gs
                    pj = ppool.tile([128, cw], BF16, tag="P",
                                    name=f"pj{h}")
                    for (po, pw) in _pieces(cw):
                        ps = qsc.tile([128, 512], F32, tag="qsc",
                                      name=f"sc{h}")
                        nc.tensor.matmul(
                            ps[:, ds(0, pw)],
                            (qk[0][4 + ht][ds(hb, 64), ds(gs, 128)]),
                            (qk[0][ht][ds(hb, 64), ds(gs + po, pw)]),
                            start=True, stop=True)
                        nc.scalar.activation(pj[:, ds(po, pw)],
                                             ps[:, ds(0, pw)],
                                             EXP, scale=SCALE)
                        if po == 0:
                            mask_diag(pj)
                    # software-pipeline: av for block j-1 sits behind
                    # block j's scores in the PE FIFO, so exp(j-1) gets a
                    # scores-duration of slack before PE waits on it
                    if prev is not None:
                        av_mm(0, h, prev[0], pav, prev[1], prev[2])
                    prev = (j, pj, gs)
                av_mm(0, h, prev[0], pav, prev[1], prev[2])
                normalize(0, ht, h, pav)

        def attn1(ht, pssc, psav):
            # second-q-half attention, one head at a time (ACT-bound
            # phase: full-width 1024-col exp calls minimize ACT overhead;
            # odd head first so its ot DMA overlaps the even head)
            for h in (2 * ht + 1, 2 * ht):
                hb = 64 * (h % 2)
                pav = psav.tile([DH + 1, 1024], F32, tag="pav")
                for j in range(16):
                    gs = max(1024, 128 * j)
                    cw = 2048 - gs
                    kt = qk[j // 8][4 + ht]
                    k_ap = kt[ds(hb, 64), ds(128 * (j % 8), 128)]
                    ps = pssc.tile([128, cw], F32, tag="sc")
                    for (po, pw) in _pieces(cw):
                        nc.tensor.matmul(
                            ps[:, ds(po, pw)], k_ap,
                            (qk[1][ht][ds(hb, 64), ds(gs - 1024 + po, pw)]),
                            start=True, stop=True)
                    pj = ppool.tile([128, cw], BF16, tag="P")
                    nc.scalar.activation(pj[:], ps[:], EXP, scale=SCALE)
                    if gs == 128 * j:
                        mask_diag(pj)
                    av_mm(1, h, j, pav, pj, gs)
                normalize(1, ht, h, pav)

        def outproj(tt, psy):
            # y[tt*128 : tt*128+128, :] — needs ot[tt // 8][k] for all k.
            # PSUM comes out of the qh1 scores ring (same shape/tag) so
            # the attention pools keep full double-buffering.
            oth = ot[tt // 8]
            ysb = ypool.tile([128, D], F32, tag="y")
            pp = psy.tile([128, 1024], F32, tag="sc")
            for k in range(4):
                for half in range(2):
                    nc.tensor.matmul(
                        pp[:, ts(half, 512)], (oth[k][:, ts(tt % 8, 128)]),
                        (wo_sb[:, k, ts(half, 512)]),
                        start=(k == 0), stop=(k == 3))
            nc.vector.tensor_copy(ysb[:], pp[:])
            nc.sync.dma_start(y[ts(tt, 128), :], ysb[:])

        # ---- stage 1: proj h0 -> attn qh0 || proj h1 ------------------
        with ExitStack() as st1:
            qsc = st1.enter_context(
                tc.tile_pool(name="qsc", bufs=4, space="PSUM"))
            pvb = st1.enter_context(
                tc.tile_pool(name="pvb", bufs=2, space="PSUM"))
            psav0 = st1.enter_context(
                tc.tile_pool(name="psav0", bufs=1, space="PSUM"))
            load_x(0)
            proj_v(0, pvb)
            proj_qk(0, 0, qsc, pvb)
            proj_qk(0, 4, qsc, pvb)
            attn0(0, qsc, psav0)
            load_x(1)
            proj_qk(0, 1, qsc, pvb)
            proj_qk(0, 5, qsc, pvb)
            proj_v(1, pvb)
            attn0(1, qsc, psav0)
            for ht in (2, 3):
                proj_qk(0, ht, qsc, pvb)
                proj_qk(0, 4 + ht, qsc, pvb)
                proj_qk(1, ht - 2, qsc, pvb)
                proj_qk(1, ht + 2, qsc, pvb)
                attn0(ht, qsc, psav0)
            for t in (2, 6, 3, 7):
                proj_qk(1, t, qsc, pvb)

        # ---- stage 2: attn qh1 || outproj -----------------------------
        with ExitStack() as st2:
            pss1 = st2.enter_context(
                tc.tile_pool(name="pss1", bufs=2, space="PSUM"))
            psav1 = st2.enter_context(
                tc.tile_pool(name="psav1", bufs=2, space="PSUM"))
            for ht in range(4):
                attn1(ht, pss1, psav1)
                outproj(2 * ht, pss1)
                outproj(2 * ht + 1, pss1)
            for tt in range(8, 16):
                outproj(tt, pss1)
    return nc


# ---------------- host side ------------------------------------------------

def _bf16(a):
    import ml_dtypes
    return np.ascontiguousarray(np.asarray(a).astype(ml_dtypes.bfloat16))


def _rope_tables():
    i = np.arange(DH // 2, dtype=np.float32)
    thetas = np.power(np.float32(10000.0), -2.0 * (i - 1.0) / DH)
    vals = thetas[:, None].astype(np.float32) * \
        np.arange(S, dtype=np.float32)[None, :]
    cos32 = np.cos(vals).astype(np.float32)
    sin32 = np.sin(vals).astype(np.float32)
    CC = np.tile(cos32, (4, 1))
    SSsw = np.concatenate([sin32, -sin32, sin32, -sin32], axis=0)
    return _bf16(CC), _bf16(SSsw)


def _perm_matrix():
    P = np.zeros((128, 128), dtype=np.float32)
    for m in range(128):
        P[m ^ 32, m] = 1.0
    return P


def _qk_col_perm(g):
    cols = []
    for m in range(4):
        for hh in (2 * m, 2 * m + 1):
            hg = HL * g + hh
            cols += [hg * DH + 2 * i for i in range(32)]
            cols += [hg * DH + 2 * i + 1 for i in range(32)]
    return np.array(cols)


_CACHE = {}


def _get_module():
    if "nc" not in _CACHE:
        nc = bacc.Bacc("TRN2", target_bir_lowering=False, debug=False,
                       num_devices=8)
        build_kernel(nc)
        nc.compile()
        _CACHE["nc"] = nc
    return _CACHE["nc"]


def make_in_maps(x, Wqkv, Wout):
    x = np.ascontiguousarray(np.asarray(x, np.float32))
    Wqkv = np.ascontiguousarray(np.asarray(Wqkv, np.float32))
    Wout = np.ascontiguousarray(np.asarray(Wout, np.float32))
    CC, SSsw = _rope_tables()
    P = _bf16(_perm_matrix())
    shard = {}
    for g in range(2):
        perm = _qk_col_perm(g)
        vcols = np.arange(HL * g * DH, HL * (g + 1) * DH)
        wqp = Wqkv[:, 0 * INNER:1 * INNER][:, perm]
        wkp = Wqkv[:, 1 * INNER:2 * INNER][:, perm]
        # [t, p, k, n]: tile t's [D, 128] block split into 8 [128, 128]
        # contraction chunks, partition-major
        wqk = np.stack([
            (wqp if t < 4 else wkp)[:, (t % 4) * 128:(t % 4 + 1) * 128]
            .reshape(KC, 128, 128).transpose(1, 0, 2)
            for t in range(8)])
        shard[g] = dict(
            wqk=_bf16(wqk),
            wv=_bf16(Wqkv[:, 2 * INNER:3 * INNER][:, vcols]
                     .reshape(KC, 128, 512).transpose(1, 0, 2)),
            wo=_bf16(Wout[vcols, :].reshape(4, 128, D).transpose(1, 0, 2)),
        )
    in_maps = []
    for c in range(8):
        b, g = c // 2, c % 2
        in_maps.append(dict(
            xT=_bf16(x[b].T), cc=CC, ssw=SSsw, perm=P, **shard[g]))
    return in_maps


def kernel(x, Wqkv, Wout, bout):
    bout = np.asarray(bout, np.float32)
    nc = _get_module()
    in_maps = make_in_maps(x, Wqkv, Wout)
    res = run_bass_kernel_spmd(nc, in_maps, core_ids=list(range(8)))
    ys = [r["y"] for r in res.results]
    out = np.stack([ys[2 * b] + ys[2 * b + 1] + bout for b in range(B)])
    return out.astype(np.float32)
